# revision 16
# baseline (speedup 1.0000x reference)
"""Gaussian KDE (bandwidth=0.5) on 8 TRN2 NeuronCores.

out[j] = sum_i mask_i * exp(-|s_i - l_j|^2 / bw^2), normalized to sum 1.

Data-parallel over samples: core c gets samples[c*2048:(c+1)*2048] and all
8192 locations. The exp argument is a K=8 bf16 matmul (hi/lo split of both
operands, f32 PSUM accumulate):

    arg[p,i] = th_i + tl_i + 8(sx_i*lx_j + sy_i*ly_j)     (j = p*64 + b)
    t_i = -4|s_i|^2 + (0 if in-bbox else -1000)

All sample/location-side prep (bf16 splits, bbox mask fold, biases) is done
on the host; the device program is just matmul + exp + reduce + all-reduce.

The per-pair exp over each [128, 2048] PSUM block is split across engines,
alternating by block (fixed in the SPMD program):
  - even blocks (A): ScalarE ACT exp with per-partition bias and fused
    free-axis accumulate -> acc[:, b].
  - odd blocks (B): DVE computes a Schraudolph-style bf16 exp: u16 =
    saturate(round(A_SCH*psum + (A_SCH*bias_j + 16256 - sigma_c))), whose
    bits are exp(y)*2^(-sigma/128) in bf16. Pool then pair-adds the bf16
    view 2048->1024->512 and DVE accumulates to f32 with the compensation
    scale 2^(sigma/128)/MU (MU = Schraudolph mean bias, calibrated).
    sigma_c = 9 + 16c is dithered per core so the mantissa-interpolation
    error averages out across the 8-core all-reduce.

AllReduce in 3 chunks overlapped with compute; normalization on-device
(partition sum via PE ones-matmul, reciprocal, broadcast).
"""

import sys

sys.path.insert(0, "/opt/trn_rl_repo")

import numpy as np

N_CORES = 8
NS = 16384
NL = 8192
NS_SH = NS // N_CORES  # 2048 samples per core
NBLK = NL // 128  # 64 location blocks
MM_N = 512  # PSUM bank limit (512 f32 outputs per matmul)
BW = 0.5
INV_BW2 = 1.0 / (BW * BW)  # 4.0
PEN = 1000.0
A_SCH = float(np.float32(128.0 / np.log(2.0)))  # Schraudolph scale
B_SCH = 16256.0  # 127 * 128
MU = 1.0407  # Schraudolph k-weighted mean bias (calibrated)
BNDS = [0, 23, 45, 64]  # all-reduce chunk boundaries (ends on A-blocks so
# the lagged B accums don't delay the chunk DMA/all-reduce issue)
N_CHUNKS = len(BNDS) - 1

_STATE = {}


def _is_a(b):
    return b % 2 == 0


def build_nc():
    import concourse.bacc as bacc
    import concourse.mybir as mybir
    import concourse.tile as tile

    f32 = mybir.dt.float32
    bf16 = mybir.dt.bfloat16
    u16 = mybir.dt.uint16
    AX = mybir.AxisListType
    AF = mybir.ActivationFunctionType
    AL = mybir.AluOpType

    nc = bacc.Bacc(None, target_bir_lowering=False, num_devices=N_CORES)

    sta_d = nc.declare_dram_parameter("sta", [8, NL], bf16, isOutput=False)
    mov_d = nc.declare_dram_parameter("mov", [8, NS_SH], bf16, isOutput=False)
    ba_d = nc.declare_dram_parameter("biasa", [128, NBLK], f32, isOutput=False)
    bb_d = nc.declare_dram_parameter("biasb", [128, NBLK], f32, isOutput=False)
    cc_d = nc.declare_dram_parameter("compc", [128, 1], f32, isOutput=False)
    out_d = nc.declare_dram_parameter("out", [128, NBLK], f32, isOutput=True)

    with tile.TileContext(nc) as tc:
        with tc.tile_pool(name="const", bufs=1) as cpool, \
             tc.tile_pool(name="dram", bufs=1, space="DRAM") as dpool, \
             tc.tile_pool(name="es", bufs=2) as epool, \
             tc.tile_pool(name="us", bufs=2) as upool, \
             tc.tile_pool(name="tr", bufs=2) as tpool, \
             tc.tile_pool(name="ps", bufs=2, space="PSUM") as ppool:

            Lb = cpool.tile([8, NL], bf16)
            Rb = cpool.tile([8, NS_SH], bf16)
            BA = cpool.tile([128, NBLK], f32)
            BB = cpool.tile([128, NBLK], f32)
            CC = cpool.tile([128, 1], f32)
            acc = cpool.tile([128, NBLK], f32)
            G = cpool.tile([128, NBLK], f32)
            Gs = cpool.tile([128, 1], f32)
            ones128 = cpool.tile([128, 1], f32)
            ones1 = cpool.tile([1, 128], f32)
            tot = cpool.tile([1, 1], f32)
            rtot = cpool.tile([1, 1], f32)
            rb = cpool.tile([128, 1], f32)

            partials = [
                dpool.tile([128, BNDS[g + 1] - BNDS[g]], f32, name=f"partial{g}")
                for g in range(N_CHUNKS)
            ]
            allsums = [
                dpool.tile(
                    [128, BNDS[g + 1] - BNDS[g]],
                    f32,
                    addr_space="Shared",
                    name=f"allsum{g}",
                )
                for g in range(N_CHUNKS)
            ]

            # ---- input loads, spread across engine queues so the
            # 16KB/partition stationary doesn't serialize on one queue ----
            QL = NL // 4
            nc.sync.dma_start(out=Rb[:, :], in_=mov_d[:, :])
            nc.scalar.dma_start(out=Lb[:, 0:QL], in_=sta_d[:, 0:QL])
            nc.gpsimd.dma_start(out=Lb[:, QL : 2 * QL], in_=sta_d[:, QL : 2 * QL])
            nc.sync.dma_start(
                out=Lb[:, 2 * QL : 3 * QL], in_=sta_d[:, 2 * QL : 3 * QL]
            )
            nc.scalar.dma_start(out=Lb[:, 3 * QL :], in_=sta_d[:, 3 * QL :])
            nc.gpsimd.dma_start(out=BA[:, :], in_=ba_d[:, :])
            nc.sync.dma_start(out=BB[:, :], in_=bb_d[:, :])
            nc.scalar.dma_start(out=CC[:, :], in_=cc_d[:, :])
            nc.gpsimd.memset(ones128[:], 1.0)
            nc.gpsimd.memset(ones1[:], 1.0)

            # ---- main loop ----
            # The B-path accum (DVE) for block b is issued after the NEXT
            # B-block's Schraudolph so the in-order DVE queue never stalls
            # waiting on the Pool tree.  Chunk all-reduces fire as soon as
            # the accum completing their last B column is issued.
            h = NS_SH // 2
            q = NS_SH // 4
            b_blocks = [b for b in range(NBLK) if not _is_a(b)]
            last_b_of_chunk = {}
            for g in range(N_CHUNKS):
                bs = [b for b in b_blocks if BNDS[g] <= b < BNDS[g + 1]]
                last_b_of_chunk[bs[-1]] = g

            def issue_chunk(g):
                lo, hi = BNDS[g], BNDS[g + 1]
                nc.sync.dma_start(out=partials[g][:, :], in_=acc[:, lo:hi])
                nc.gpsimd.collective_compute(
                    "AllReduce",
                    AL.add,
                    replica_groups=[list(range(N_CORES))],
                    ins=[partials[g][:, :]],
                    outs=[allsums[g][:, :]],
                )

            def issue_accum(pb, pt2):
                t3 = tpool.tile([128, q], bf16, tag="t3")
                nc.vector.tensor_scalar(
                    t3[:],
                    pt2[:],
                    CC[:, 0:1],
                    0.0,
                    AL.mult,
                    AL.add,
                    accum_out=acc[:, pb : pb + 1],
                )
                if pb in last_b_of_chunk:
                    issue_chunk(last_b_of_chunk[pb])

            pending = None
            for b in range(NBLK):
                ps = ppool.tile([128, NS_SH], f32, tag="ps")
                for n in range(NS_SH // MM_N):
                    nc.tensor.matmul(
                        ps[:, n * MM_N : (n + 1) * MM_N],
                        lhsT=Lb[:, b * 128 : (b + 1) * 128],
                        rhs=Rb[:, n * MM_N : (n + 1) * MM_N],
                        start=True,
                        stop=True,
                    )
                if _is_a(b):
                    es = epool.tile([128, NS_SH], bf16, tag="es")
                    nc.scalar.activation(
                        es[:],
                        ps[:],
                        AF.Exp,
                        bias=BA[:, b : b + 1],
                        scale=1.0,
                        accum_out=acc[:, b : b + 1],
                    )
                else:
                    us = upool.tile([128, NS_SH], u16, tag="us")
                    nc.vector.tensor_scalar(
                        us[:], ps[:], A_SCH, BB[:, b : b + 1], AL.mult, AL.add
                    )
                    if pending is not None:
                        issue_accum(*pending)
                        pending = None
                    V = us[:].bitcast(bf16)
                    t1 = tpool.tile([128, h], bf16, tag="t1")
                    t2 = tpool.tile([128, q], bf16, tag="t2")
                    nc.gpsimd.tensor_tensor(t1[:], V[:, 0:h], V[:, h : 2 * h], AL.add)
                    nc.gpsimd.tensor_tensor(
                        t2[:], t1[:, 0:q], t1[:, q : 2 * q], AL.add
                    )
                    pending = (b, t2)
            if pending is not None:
                issue_accum(*pending)
                pending = None

            # ---- normalize on-device ----
            for g in range(N_CHUNKS):
                nc.sync.dma_start(
                    out=G[:, BNDS[g] : BNDS[g + 1]], in_=allsums[g][:, :]
                )
            nc.vector.tensor_reduce(Gs[:], G[:], axis=AX.X, op=AL.add)
            pst = ppool.tile([1, 1], f32, tag="ps")
            nc.tensor.matmul(
                pst[:], lhsT=Gs[:], rhs=ones128[:], start=True, stop=True
            )
            nc.scalar.copy(tot[:], pst[:])
            nc.vector.reciprocal(rtot[:], tot[:])
            psb = ppool.tile([128, 1], f32, tag="ps")
            nc.tensor.matmul(
                psb[:], lhsT=ones1[:], rhs=rtot[:], start=True, stop=True
            )
            nc.scalar.copy(rb[:], psb[:])
            nc.vector.tensor_scalar(G[:], G[:], rb[:, 0:1], None, AL.mult)
            nc.sync.dma_start(out=out_d[:, :], in_=G[:])

    nc.compile()
    return nc


def _blockperm(arr):
    """arr[j] -> column q = b*128 + p where j = p*64 + b."""
    return np.ascontiguousarray(arr.reshape(128, NBLK).T.reshape(NL))


def _split(v):
    from ml_dtypes import bfloat16

    h = v.astype(bfloat16)
    l = (v - h.astype(np.float32)).astype(bfloat16)
    return h, l


def make_in_maps(samples, locations):
    from ml_dtypes import bfloat16

    lx = locations[:, 0].astype(np.float32)
    ly = locations[:, 1].astype(np.float32)
    alm = np.max(np.abs(locations), axis=0)  # [2] bbox bounds

    lxp = _blockperm(lx)
    lyp = _blockperm(ly)
    lxh, lxl = _split(lxp)
    lyh, lyl = _split(lyp)
    ones = np.ones(NL, dtype=bfloat16)
    sta = np.ascontiguousarray(
        np.stack([ones, ones, lxh, lxh, lyh, lyh, lxl, lyl])
    )

    bias = -INV_BW2 * (lx * lx + ly * ly)  # [NL] f32
    biasa = np.ascontiguousarray(
        bias.reshape(128, NBLK).astype(np.float32)
    )  # [p, b] with j = p*64 + b

    in_maps = []
    for c in range(N_CORES):
        sh = samples[c * NS_SH : (c + 1) * NS_SH]
        sx = sh[:, 0].astype(np.float32)
        sy = sh[:, 1].astype(np.float32)
        mask = np.all(np.abs(sh) < alm, axis=-1)
        t = (-INV_BW2 * (sx * sx + sy * sy) + np.where(mask, 0.0, -PEN)).astype(
            np.float32
        )
        th, tl = _split(t)
        s8xh, s8xl = _split(8.0 * sx)
        s8yh, s8yl = _split(8.0 * sy)
        mov = np.ascontiguousarray(
            np.stack([th, tl, s8xh, s8xl, s8yh, s8yl, s8xh, s8yh])
        )
        sigma = np.float32(9.0 + 16.0 * c)
        biasb = (
            np.float32(A_SCH) * biasa + (np.float32(B_SCH) - sigma)
        ).astype(np.float32)
        compc = np.full(
            (128, 1), (2.0 ** (sigma / 128.0)) / MU, dtype=np.float32
        )
        in_maps.append(
            {
                "sta": sta,
                "mov": mov,
                "biasa": biasa,
                "biasb": biasb,
                "compc": compc,
            }
        )
    return in_maps


def kernel(samples, locations):
    samples = np.ascontiguousarray(np.asarray(samples, dtype=np.float32))
    locations = np.ascontiguousarray(np.asarray(locations, dtype=np.float32))
    assert samples.shape == (NS, 2) and locations.shape == (NL, 2)

    from concourse.bass_utils import run_bass_kernel_spmd

    if "nc" not in _STATE:
        _STATE["nc"] = build_nc()
    nc = _STATE["nc"]

    in_maps = make_in_maps(samples, locations)
    res = run_bass_kernel_spmd(
        nc,
        in_maps,
        list(range(N_CORES)),
        trace=bool(_STATE.get("trace", False)),
    )
    _STATE["exec_time_ns"] = res.exec_time_ns
    _STATE["profile_json"] = res.profile_json
    return np.asarray(res.results[0]["out"], dtype=np.float32).reshape(NL)
